# revision 28
# baseline (speedup 1.0000x reference)
"""GNN message passing on 8 Trainium2 cores. v8: token-merge + wide groups.

out[n] = sum_{e : dst[e]==n} e_att[e] * src_emb[src[e]]

Same proven pipeline geometry as v6.4 (512-slot 4-block groups, 16-block
chunks, 4 SWDGE queues) but with sigma row-pairing: rows co-used by a group
share one 256B pair-token (even/odd halves weighted independently by the
A matrix), cutting gather descriptors ~20%. Groups hold up to 64 nodes
(lhsT [128, 128] = [A_ev | A_od]); psum [128, 128] per group: rows 0:64 x
cols 0:64 even part, rows 64:128 x cols 64:128 odd part; DVE copy+add
folds them into stage[128, 64] per 2-group supertile.
"""

import numpy as np

N_SRC = 50000
N_DST = 50000
D = 64
N_CORES = 8
P = 128
NPAIR = 25088
GROUP_SLOTS = 512
GROUP_NODES = 64
BLOCKS_PER_GROUP = 4
CHUNK_BLOCKS = 16
PRIME_BLOCKS = 4

_cache: dict = {}

TRACE = False
TRACE_DIR = None
LAST_EXEC_NS = None


def _wrap_idx(idx_flat):
    w = idx_flat.reshape(-1, 16).T
    return np.tile(w, (8, 1))


def _plan_core(nodes_c, deg, dst_idx, src_idx, edge_ids_c):
    eo = edge_ids_c[np.argsort(dst_idx[edge_ids_c], kind="stable")]
    ds = dst_idx[eo]
    ss = src_idx[eo]
    starts = np.searchsorted(ds, nodes_c)
    ends = np.searchsorted(ds, nodes_c, side="right")
    node_rows = {int(n): ss[a:b] for n, a, b in zip(nodes_c, starts, ends)}

    nodes_sorted = nodes_c[np.argsort(-deg[nodes_c], kind="stable")]

    tok_of_row = {}
    half_of_row = {}
    state = {"n_tok": 0, "pending": None}

    seen = set()
    groups = []
    group_rows = []
    cur_rows = set()
    cur_nodes = []

    def cur_slots(rows):
        ext = sum(1 for r in rows if r in seen)
        fa = len(rows) - ext
        return ext + (fa + 1) // 2

    def close_group():
        nonlocal cur_rows, cur_nodes
        fa = [r for r in cur_rows if r not in seen]
        i = 0
        while i + 1 < len(fa):
            r1, r2 = fa[i], fa[i + 1]
            tok_of_row[r1] = state["n_tok"]
            half_of_row[r1] = 0
            tok_of_row[r2] = state["n_tok"]
            half_of_row[r2] = 1
            state["n_tok"] += 1
            i += 2
        if i < len(fa):
            r = fa[i]
            if state["pending"] is not None:
                tok_of_row[r] = tok_of_row[state["pending"]]
                half_of_row[r] = 1
                state["pending"] = None
            else:
                tok_of_row[r] = state["n_tok"]
                half_of_row[r] = 0
                state["pending"] = r
                state["n_tok"] += 1
        seen.update(fa)
        groups.append(cur_nodes)
        group_rows.append(cur_rows)
        cur_rows = set()
        cur_nodes = []

    for n in nodes_sorted:
        rows = set(int(r) for r in node_rows[int(n)])
        test = cur_rows | rows
        if cur_nodes and (
            len(cur_nodes) + 1 > GROUP_NODES or cur_slots(test) > GROUP_SLOTS
        ):
            close_group()
            test = rows
        cur_rows = test
        cur_nodes.append(int(n))
    if cur_nodes:
        close_group()

    slot_tok = []
    for rows in group_rows:
        toks = sorted({tok_of_row[r] for r in rows})
        assert len(toks) <= GROUP_SLOTS
        slot_tok.append(toks)
    return groups, slot_tok, tok_of_row, half_of_row


def _plan(src_idx, dst_idx, att):
    deg = np.bincount(dst_idx, minlength=N_DST)
    order = np.argsort(-deg, kind="stable")
    core_of_node = np.zeros(N_DST, dtype=np.int64)
    for i, n in enumerate(order):
        blk, pos = divmod(i, N_CORES)
        core_of_node[n] = pos if blk % 2 == 0 else N_CORES - 1 - pos

    core_e = core_of_node[dst_idx]
    plans = []
    for c in range(N_CORES):
        nodes_c = np.flatnonzero((core_of_node == c) & (deg > 0))
        edge_ids_c = np.flatnonzero(core_e == c)
        plans.append(_plan_core(nodes_c, deg, dst_idx, src_idx, edge_ids_c))

    G = max(len(p[0]) for p in plans)
    G = -(-G // 2) * 2
    NB = G * BLOCKS_PER_GROUP
    NS = NB * P

    idx2 = np.zeros((N_CORES, NS), dtype=np.int16)
    a3 = np.zeros((N_CORES, NS, 2 * GROUP_NODES), dtype=np.float32)
    node_at = np.full((N_CORES, G, GROUP_NODES), -1, dtype=np.int64)
    sigma = np.zeros((N_CORES, NPAIR, 2), dtype=np.int64)

    for c in range(N_CORES):
        groups, slot_tok, tok_of_row, half_of_row = plans[c]
        used = np.zeros(NPAIR * 2, dtype=bool)
        rowfill = np.zeros((NPAIR, 2), dtype=np.int64)
        for r, t in tok_of_row.items():
            h = half_of_row[r]
            rowfill[t, h] = r
            used[t * 2 + h] = True
        free_slots = np.flatnonzero(~used)
        allrows = np.ones(NPAIR * 2, dtype=bool)
        refd = np.array(list(tok_of_row.keys()), dtype=np.int64)
        if len(refd):
            allrows[refd] = False
        leftover = np.flatnonzero(allrows)
        ns = min(len(free_slots), len(leftover))
        rowfill.reshape(-1)[free_slots[:ns]] = leftover[:ns]
        sigma[c] = rowfill

        col_of = {}
        for g, members in enumerate(groups):
            for j, n in enumerate(members):
                node_at[c, g, j] = n
                col_of[n] = (g, j)
        slotidx = {}
        for g, toks in enumerate(slot_tok):
            base = g * GROUP_SLOTS
            for k, t in enumerate(toks):
                idx2[c, base + k] = t
                slotidx[(g, t)] = base + k
        eids = np.flatnonzero(core_e == c)
        s_slots = np.empty(len(eids), dtype=np.int64)
        s_cols = np.empty(len(eids), dtype=np.int64)
        for k, e in enumerate(eids):
            r = int(src_idx[e])
            g, j = col_of[int(dst_idx[e])]
            s_slots[k] = slotidx[(g, tok_of_row[r])]
            s_cols[k] = j + GROUP_NODES * half_of_row[r]
        np.add.at(a3[c], (s_slots, s_cols), att[eids])

    chunks = []
    b0 = 0
    while b0 < NB:
        nb = PRIME_BLOCKS if len(chunks) < 4 else CHUNK_BLOCKS
        nb = min(nb, NB - b0)
        chunks.append((b0, nb))
        b0 += nb

    return {
        "NB": NB,
        "G": G,
        "chunks": tuple(chunks),
        "idx2": idx2,
        "a3": a3.astype(np.float16),
        "node_at": node_at,
        "sigma": sigma,
        "pad_frac": 1.0 - len(dst_idx) / (N_CORES * NS),
    }


def _build_nc(NB, chunks):
    import concourse.bacc as bacc
    import concourse.mybir as mybir
    from concourse.tile import TileContext
    from concourse.library_config import mlp

    NS = NB * P
    nsuper = NB // 8

    nc = bacc.Bacc(
        "TRN2", target_bir_lowering=False, debug=False, num_swdge_queues=4,
        dynamic_dma_scratch_size=65536,
    )
    embP = nc.dram_tensor("embP", [NPAIR, P], mybir.dt.float16, kind="ExternalInput")
    idxT = nc.dram_tensor("idxT", [P, NS // 16], mybir.dt.int16, kind="ExternalInput")
    atab = nc.dram_tensor("atab", [P, NB * 128], mybir.dt.float16, kind="ExternalInput")
    out = nc.dram_tensor("out", [nsuper * P, D], mybir.dt.float32, kind="ExternalOutput")

    with TileContext(nc) as tc:
        nc.gpsimd.load_library(mlp)
        with (
            tc.tile_pool(name="tbl", bufs=1) as tbl,
            tc.tile_pool(name="msg", bufs=12) as msgp,
            tc.tile_pool(name="apool", bufs=6) as apool,
            tc.tile_pool(name="psum", bufs=8, space="PSUM") as psump,
            tc.tile_pool(name="stg", bufs=6) as stgp,
        ):
            head_blocks = sum(nb for _, nb in chunks[:5])
            head_cols = head_blocks * 8
            tail_cols = NS // 16 - head_cols
            idx_a = tbl.tile([P, head_cols], mybir.dt.int16, tag="idxa")
            nc.sync.dma_start(idx_a[:], idxT[:, :head_cols])
            if tail_cols > 0:
                idx_b = tbl.tile([P, tail_cols], mybir.dt.int16, tag="idxb")
                nc.sync.dma_start(idx_b[:], idxT[:, head_cols:])

            psum_tiles = {}
            stage_tiles = {}
            for ci, (b0, nb) in enumerate(chunks):
                q = ci % 4
                c_lo, c_hi = b0 * 8, (b0 + nb) * 8
                if c_hi <= head_cols:
                    iap = idx_a[:, c_lo:c_hi]
                else:
                    iap = idx_b[:, c_lo - head_cols : c_hi - head_cols]
                nidx = nb * P
                msg = msgp.tile([P, CHUNK_BLOCKS, P], mybir.dt.float16, tag="m")
                nc.gpsimd.dma_gather(
                    msg[:, :nb, :], embP[:, :],
                    iap, nidx, nidx, P,
                    transpose=False, single_packet=False, queue_num=q,
                )
                a_t = apool.tile([P, CHUNK_BLOCKS * 128], mybir.dt.float16, tag="a")
                nc.scalar.dma_start(
                    a_t[:, : nb * 128], atab[:, b0 * 128 : (b0 + nb) * 128]
                )

                for j in range(nb):
                    b = b0 + j
                    g = b // BLOCKS_PER_GROUP
                    st = b // (2 * BLOCKS_PER_GROUP)
                    gl = g % 2
                    if g not in psum_tiles:
                        psum_tiles[g] = psump.tile(
                            [P, P], mybir.dt.float32, tag="ps", name=f"ps{g}"
                        )
                    ps = psum_tiles[g]
                    nc.tensor.matmul(
                        ps[:, :], a_t[:, j * 128 : j * 128 + 128],
                        msg[:, j, :],
                        start=(b % BLOCKS_PER_GROUP == 0),
                        stop=(b % BLOCKS_PER_GROUP == BLOCKS_PER_GROUP - 1),
                    )
                    if b % BLOCKS_PER_GROUP == BLOCKS_PER_GROUP - 1:
                        if gl == 0:
                            stage_tiles[st] = stgp.tile(
                                [P, D], mybir.dt.float32, tag="st", name=f"st{st}"
                            )
                        stage = stage_tiles[st]
                        nc.vector.tensor_copy(
                            stage[64 * gl : 64 * gl + 64, :],
                            ps[0:64, 0:D],
                        )
                        nc.vector.tensor_tensor(
                            stage[64 * gl : 64 * gl + 64, :],
                            stage[64 * gl : 64 * gl + 64, :],
                            ps[64:128, D : 2 * D],
                            mybir.AluOpType.add,
                        )
                        del psum_tiles[g]
                        if gl == 1:
                            nc.sync.dma_start(
                                out[st * P : (st + 1) * P, :], stage[:, :]
                            )
                            del stage_tiles[st]
    nc.compile()
    return nc


def plan_and_build(src_idx, dst_idx, e_att):
    src_idx = np.asarray(src_idx, dtype=np.int64)
    dst_idx = np.asarray(dst_idx, dtype=np.int64)
    att_flat = np.asarray(e_att, dtype=np.float32).reshape(-1)
    return _plan(src_idx, dst_idx, att_flat)


def kernel(src_emb, e_att, src_idx, dst_idx):
    from concourse.bass_utils import run_bass_kernel_spmd

    src_emb = np.asarray(src_emb, dtype=np.float32)
    pl = plan_and_build(src_idx, dst_idx, e_att)

    key = (pl["NB"], pl["chunks"])
    if key not in _cache:
        _cache.clear()
        _cache[key] = _build_nc(pl["NB"], pl["chunks"])
    nc = _cache[key]

    emb16 = np.zeros((NPAIR * 2, D), dtype=np.float16)
    emb16[:N_SRC] = src_emb.astype(np.float16)

    NB = pl["NB"]
    in_maps = []
    for c in range(N_CORES):
        embPc = emb16[pl["sigma"][c].reshape(-1)].reshape(NPAIR, P)
        at = np.ascontiguousarray(
            pl["a3"][c].reshape(NB, P, 128).transpose(1, 0, 2).reshape(P, NB * 128)
        )
        in_maps.append(
            {
                "embP": np.ascontiguousarray(embPc),
                "idxT": np.ascontiguousarray(_wrap_idx(pl["idx2"][c].reshape(-1))),
                "atab": at,
            }
        )
    kwargs = {}
    if TRACE:
        kwargs = {"trace": True, "tmpdir": TRACE_DIR}
    res = run_bass_kernel_spmd(nc, in_maps, core_ids=list(range(N_CORES)), **kwargs)
    global LAST_EXEC_NS
    LAST_EXEC_NS = res.exec_time_ns

    out_full = np.zeros((N_DST, D), dtype=np.float32)
    G = pl["G"]
    node_at = pl["node_at"]  # [ncores, G, 64]
    for c in range(N_CORES):
        ids = node_at[c].reshape(-1)
        valid = ids >= 0
        out_full[ids[valid]] = res.results[c]["out"][valid]
    return out_full


# revision 31
# speedup vs baseline: 1.1406x; 1.1406x over previous
"""GNN message passing on 8 Trainium2 cores. v8: token-merge + wide groups.

out[n] = sum_{e : dst[e]==n} e_att[e] * src_emb[src[e]]

Same proven pipeline geometry as v6.4 (512-slot 4-block groups, 16-block
chunks, 4 SWDGE queues) but with sigma row-pairing: rows co-used by a group
share one 256B pair-token (even/odd halves weighted independently by the
A matrix), cutting gather descriptors ~20%. Groups hold up to 64 nodes
(lhsT [128, 128] = [A_ev | A_od]); psum [128, 128] per group: rows 0:64 x
cols 0:64 even part, rows 64:128 x cols 64:128 odd part; DVE copy+add
folds them into stage[128, 64] per 2-group supertile.
"""

import numpy as np

N_SRC = 50000
N_DST = 50000
D = 64
N_CORES = 8
P = 128
NPAIR = 25088
GROUP_SLOTS = 512
GROUP_NODES = 64
BLOCKS_PER_GROUP = 4
CHUNK_BLOCKS = 16
PRIME_BLOCKS = 4

_cache: dict = {}

TRACE = False
TRACE_DIR = None
LAST_EXEC_NS = None


def _wrap_idx(idx_flat):
    w = idx_flat.reshape(-1, 16).T
    return np.tile(w, (8, 1))


def _plan_core(nodes_c, deg, dst_idx, src_idx, edge_ids_c):
    eo = edge_ids_c[np.argsort(dst_idx[edge_ids_c], kind="stable")]
    ds = dst_idx[eo]
    ss = src_idx[eo]
    starts = np.searchsorted(ds, nodes_c)
    ends = np.searchsorted(ds, nodes_c, side="right")
    node_rows = {int(n): ss[a:b] for n, a, b in zip(nodes_c, starts, ends)}

    nodes_sorted = nodes_c[np.argsort(-deg[nodes_c], kind="stable")]

    tok_of_row = {}
    half_of_row = {}
    state = {"n_tok": 0, "pending": None}

    seen = set()
    groups = []
    group_rows = []
    group_fa = []
    cur_rows = set()
    cur_nodes = []

    def cur_slots(rows):
        ext = sum(1 for r in rows if r in seen)
        fa = len(rows) - ext
        return ext + (fa + 1) // 2

    def close_group():
        nonlocal cur_rows, cur_nodes
        fa = [r for r in cur_rows if r not in seen]
        seen.update(fa)
        groups.append(cur_nodes)
        group_rows.append(cur_rows)
        group_fa.append(fa)
        cur_rows = set()
        cur_nodes = []

    for n in nodes_sorted:
        rows = set(int(r) for r in node_rows[int(n)])
        test = cur_rows | rows
        if cur_nodes and (
            len(cur_nodes) + 1 > GROUP_NODES or cur_slots(test) > GROUP_SLOTS
        ):
            close_group()
            test = rows
        cur_rows = test
        cur_nodes.append(int(n))
    if cur_nodes:
        close_group()

    # pass 2: sigma pairing; among each group's first-appearance rows prefer
    # pairs co-occurring in a SECOND group (their token then serves both).
    from collections import defaultdict
    row_grps = defaultdict(list)
    for g, rows in enumerate(group_rows):
        for r in rows:
            row_grps[r].append(g)

    def assign_pair(r1, r2):
        tok_of_row[r1] = state["n_tok"]
        half_of_row[r1] = 0
        tok_of_row[r2] = state["n_tok"]
        half_of_row[r2] = 1
        state["n_tok"] += 1

    for g, fa in enumerate(group_fa):
        remaining = set(fa)
        bysig = defaultdict(list)
        for r in fa:
            for og in row_grps[r]:
                if og > g:
                    bysig[og].append(r)
        for og, rs in sorted(bysig.items()):
            rs = [r for r in rs if r in remaining]
            for i in range(0, len(rs) - 1, 2):
                assign_pair(rs[i], rs[i + 1])
                remaining.discard(rs[i])
                remaining.discard(rs[i + 1])
        rest = sorted(remaining)
        i = 0
        while i + 1 < len(rest):
            assign_pair(rest[i], rest[i + 1])
            i += 2
        if i < len(rest):
            r = rest[i]
            if state["pending"] is not None:
                tok_of_row[r] = tok_of_row[state["pending"]]
                half_of_row[r] = 1
                state["pending"] = None
            else:
                tok_of_row[r] = state["n_tok"]
                half_of_row[r] = 0
                state["pending"] = r
                state["n_tok"] += 1

    # pass 3: REPACK groups by actual unique-token count so the shared
    # tokens convert into fewer groups (not pad)
    node_toks = {
        int(n): set(tok_of_row[int(r)] for r in node_rows[int(n)])
        for n in nodes_c
    }
    groups = []
    group_toks = []
    cur_toks = set()
    cur_nodes = []
    for n in nodes_sorted:
        test = cur_toks | node_toks[int(n)]
        if cur_nodes and (
            len(cur_nodes) + 1 > GROUP_NODES or len(test) > GROUP_SLOTS
        ):
            groups.append(cur_nodes)
            group_toks.append(cur_toks)
            cur_toks = set()
            cur_nodes = []
            test = set(node_toks[int(n)])
        cur_toks = test
        cur_nodes.append(int(n))
    if cur_nodes:
        groups.append(cur_nodes)
        group_toks.append(cur_toks)

    slot_tok = []
    for toks in group_toks:
        ts = sorted(toks)
        assert len(ts) <= GROUP_SLOTS
        slot_tok.append(ts)
    return groups, slot_tok, tok_of_row, half_of_row


def _plan(src_idx, dst_idx, att):
    deg = np.bincount(dst_idx, minlength=N_DST)
    order = np.argsort(-deg, kind="stable")
    core_of_node = np.zeros(N_DST, dtype=np.int64)
    for i, n in enumerate(order):
        blk, pos = divmod(i, N_CORES)
        core_of_node[n] = pos if blk % 2 == 0 else N_CORES - 1 - pos

    core_e = core_of_node[dst_idx]
    plans = []
    for c in range(N_CORES):
        nodes_c = np.flatnonzero((core_of_node == c) & (deg > 0))
        edge_ids_c = np.flatnonzero(core_e == c)
        plans.append(_plan_core(nodes_c, deg, dst_idx, src_idx, edge_ids_c))

    G = max(len(p[0]) for p in plans)
    G = -(-G // 2) * 2
    NB = G * BLOCKS_PER_GROUP
    NS = NB * P

    idx2 = np.zeros((N_CORES, NS), dtype=np.int16)
    a3 = np.zeros((N_CORES, NS, 2 * GROUP_NODES), dtype=np.float32)
    node_at = np.full((N_CORES, G, GROUP_NODES), -1, dtype=np.int64)
    sigma = np.zeros((N_CORES, NPAIR, 2), dtype=np.int64)

    for c in range(N_CORES):
        groups, slot_tok, tok_of_row, half_of_row = plans[c]
        used = np.zeros(NPAIR * 2, dtype=bool)
        rowfill = np.zeros((NPAIR, 2), dtype=np.int64)
        for r, t in tok_of_row.items():
            h = half_of_row[r]
            rowfill[t, h] = r
            used[t * 2 + h] = True
        free_slots = np.flatnonzero(~used)
        allrows = np.ones(NPAIR * 2, dtype=bool)
        refd = np.array(list(tok_of_row.keys()), dtype=np.int64)
        if len(refd):
            allrows[refd] = False
        leftover = np.flatnonzero(allrows)
        ns = min(len(free_slots), len(leftover))
        rowfill.reshape(-1)[free_slots[:ns]] = leftover[:ns]
        sigma[c] = rowfill

        col_of = {}
        for g, members in enumerate(groups):
            for j, n in enumerate(members):
                node_at[c, g, j] = n
                col_of[n] = (g, j)
        slotidx = {}
        for g, toks in enumerate(slot_tok):
            base = g * GROUP_SLOTS
            for k, t in enumerate(toks):
                idx2[c, base + k] = t
                slotidx[(g, t)] = base + k
        eids = np.flatnonzero(core_e == c)
        s_slots = np.empty(len(eids), dtype=np.int64)
        s_cols = np.empty(len(eids), dtype=np.int64)
        for k, e in enumerate(eids):
            r = int(src_idx[e])
            g, j = col_of[int(dst_idx[e])]
            s_slots[k] = slotidx[(g, tok_of_row[r])]
            s_cols[k] = j + GROUP_NODES * half_of_row[r]
        np.add.at(a3[c], (s_slots, s_cols), att[eids])

    chunks = []
    b0 = 0
    while b0 < NB:
        nb = PRIME_BLOCKS if len(chunks) < 4 else CHUNK_BLOCKS
        nb = min(nb, NB - b0)
        chunks.append((b0, nb))
        b0 += nb

    return {
        "NB": NB,
        "G": G,
        "chunks": tuple(chunks),
        "idx2": idx2,
        "a3": a3.astype(np.float16),
        "node_at": node_at,
        "sigma": sigma,
        "pad_frac": 1.0 - len(dst_idx) / (N_CORES * NS),
    }


def _build_nc(NB, chunks):
    import concourse.bacc as bacc
    import concourse.mybir as mybir
    from concourse.tile import TileContext
    from concourse.library_config import mlp

    NS = NB * P
    nsuper = NB // 8

    nc = bacc.Bacc(
        "TRN2", target_bir_lowering=False, debug=False, num_swdge_queues=4,
        dynamic_dma_scratch_size=65536,
    )
    embP = nc.dram_tensor("embP", [NPAIR, P], mybir.dt.float16, kind="ExternalInput")
    idxT = nc.dram_tensor("idxT", [P, NS // 16], mybir.dt.int16, kind="ExternalInput")
    atab = nc.dram_tensor("atab", [P, NB * 128], mybir.dt.float16, kind="ExternalInput")
    out = nc.dram_tensor("out", [nsuper * P, D], mybir.dt.float32, kind="ExternalOutput")

    with TileContext(nc) as tc:
        nc.gpsimd.load_library(mlp)
        with (
            tc.tile_pool(name="tbl", bufs=1) as tbl,
            tc.tile_pool(name="msg", bufs=12) as msgp,
            tc.tile_pool(name="apool", bufs=6) as apool,
            tc.tile_pool(name="psum", bufs=8, space="PSUM") as psump,
            tc.tile_pool(name="stg", bufs=6) as stgp,
        ):
            head_blocks = sum(nb for _, nb in chunks[:5])
            head_cols = head_blocks * 8
            tail_cols = NS // 16 - head_cols
            idx_a = tbl.tile([P, head_cols], mybir.dt.int16, tag="idxa")
            nc.sync.dma_start(idx_a[:], idxT[:, :head_cols])
            if tail_cols > 0:
                idx_b = tbl.tile([P, tail_cols], mybir.dt.int16, tag="idxb")
                nc.sync.dma_start(idx_b[:], idxT[:, head_cols:])

            psum_tiles = {}
            stage_tiles = {}
            for ci, (b0, nb) in enumerate(chunks):
                q = ci % 4
                c_lo, c_hi = b0 * 8, (b0 + nb) * 8
                if c_hi <= head_cols:
                    iap = idx_a[:, c_lo:c_hi]
                else:
                    iap = idx_b[:, c_lo - head_cols : c_hi - head_cols]
                nidx = nb * P
                msg = msgp.tile([P, CHUNK_BLOCKS, P], mybir.dt.float16, tag="m")
                nc.gpsimd.dma_gather(
                    msg[:, :nb, :], embP[:, :],
                    iap, nidx, nidx, P,
                    transpose=False, single_packet=False, queue_num=q,
                )
                a_t = apool.tile([P, CHUNK_BLOCKS * 128], mybir.dt.float16, tag="a")
                nc.scalar.dma_start(
                    a_t[:, : nb * 128], atab[:, b0 * 128 : (b0 + nb) * 128]
                )

                for j in range(nb):
                    b = b0 + j
                    g = b // BLOCKS_PER_GROUP
                    st = b // (2 * BLOCKS_PER_GROUP)
                    gl = g % 2
                    if g not in psum_tiles:
                        psum_tiles[g] = psump.tile(
                            [P, P], mybir.dt.float32, tag="ps", name=f"ps{g}"
                        )
                    ps = psum_tiles[g]
                    nc.tensor.matmul(
                        ps[:, :], a_t[:, j * 128 : j * 128 + 128],
                        msg[:, j, :],
                        start=(b % BLOCKS_PER_GROUP == 0),
                        stop=(b % BLOCKS_PER_GROUP == BLOCKS_PER_GROUP - 1),
                    )
                    if b % BLOCKS_PER_GROUP == BLOCKS_PER_GROUP - 1:
                        if gl == 0:
                            stage_tiles[st] = stgp.tile(
                                [P, D], mybir.dt.float32, tag="st", name=f"st{st}"
                            )
                        stage = stage_tiles[st]
                        nc.vector.tensor_copy(
                            stage[64 * gl : 64 * gl + 64, :],
                            ps[0:64, 0:D],
                        )
                        nc.vector.tensor_tensor(
                            stage[64 * gl : 64 * gl + 64, :],
                            stage[64 * gl : 64 * gl + 64, :],
                            ps[64:128, D : 2 * D],
                            mybir.AluOpType.add,
                        )
                        del psum_tiles[g]
                        if gl == 1:
                            nc.sync.dma_start(
                                out[st * P : (st + 1) * P, :], stage[:, :]
                            )
                            del stage_tiles[st]
    nc.compile()
    return nc


def plan_and_build(src_idx, dst_idx, e_att):
    src_idx = np.asarray(src_idx, dtype=np.int64)
    dst_idx = np.asarray(dst_idx, dtype=np.int64)
    att_flat = np.asarray(e_att, dtype=np.float32).reshape(-1)
    return _plan(src_idx, dst_idx, att_flat)


def kernel(src_emb, e_att, src_idx, dst_idx):
    from concourse.bass_utils import run_bass_kernel_spmd

    src_emb = np.asarray(src_emb, dtype=np.float32)
    pl = plan_and_build(src_idx, dst_idx, e_att)

    key = (pl["NB"], pl["chunks"])
    if key not in _cache:
        _cache.clear()
        _cache[key] = _build_nc(pl["NB"], pl["chunks"])
    nc = _cache[key]

    emb16 = np.zeros((NPAIR * 2, D), dtype=np.float16)
    emb16[:N_SRC] = src_emb.astype(np.float16)

    NB = pl["NB"]
    in_maps = []
    for c in range(N_CORES):
        embPc = emb16[pl["sigma"][c].reshape(-1)].reshape(NPAIR, P)
        at = np.ascontiguousarray(
            pl["a3"][c].reshape(NB, P, 128).transpose(1, 0, 2).reshape(P, NB * 128)
        )
        in_maps.append(
            {
                "embP": np.ascontiguousarray(embPc),
                "idxT": np.ascontiguousarray(_wrap_idx(pl["idx2"][c].reshape(-1))),
                "atab": at,
            }
        )
    kwargs = {}
    if TRACE:
        kwargs = {"trace": True, "tmpdir": TRACE_DIR}
    res = run_bass_kernel_spmd(nc, in_maps, core_ids=list(range(N_CORES)), **kwargs)
    global LAST_EXEC_NS
    LAST_EXEC_NS = res.exec_time_ns

    out_full = np.zeros((N_DST, D), dtype=np.float32)
    G = pl["G"]
    node_at = pl["node_at"]  # [ncores, G, 64]
    for c in range(N_CORES):
        ids = node_at[c].reshape(-1)
        valid = ids >= 0
        out_full[ids[valid]] = res.results[c]["out"][valid]
    return out_full
